# revision 1
# baseline (speedup 1.0000x reference)
"""GridAttention TRN2 kernel: 8-core SPMD Bass implementation.

Sharding: core i handles batch b=i//4, output row band j=i%4 (rows 32j..32j+32).
Each core reads its batch's full x (bf16), computes conv1 -> linearized grid
attention (2nd-order softmax Taylor feature map) -> residual conv stack -> FF,
and writes its 32-row f32 output band.  All per-core variation is host data
(band slices, selection weights baked into the y-conv stationary); the device
program is identical on every core.
"""
import numpy as np
import ml_dtypes
import os
STAGE = int(os.environ.get("KSTAGE", "9"))
KSUB = int(os.environ.get("KSUB", "9"))

import concourse.bass as bass
import concourse.bacc as bacc
import concourse.tile as tile
from concourse import mybir
from concourse.bass_utils import run_bass_kernel_spmd

B, C, H, W = 2, 64, 128, 128
S = np.float32(32.0 ** 0.5)
NROW = 37
NPX = NROW * 128
PITCH = 132
F32 = mybir.dt.float32
BF16 = mybir.dt.bfloat16
BF = ml_dtypes.bfloat16

_CACHED = {}


def _split_multi_waits(nc, max_waits=1):
    """This neuronxcc accepts only ONE sync-wait per instruction; Tile emits
    several.  Move extras onto InstNoOps inserted just before, same engine."""
    f = nc.m.functions[0]
    for bb in f.blocks:
        insts = list(bb.instructions)
        new, changed = [], False
        for inst in insts:
            si = getattr(inst, "sync_info", None)
            if si is not None and si.on_wait and len(si.on_wait) > max_waits:
                waits = list(si.on_wait)
                extra, keep = waits[:-max_waits], waits[-max_waits:]
                for w in extra:
                    nop = mybir.InstNoOp(
                        name=nc.get_next_instruction_name(),
                        engine=inst.engine,
                        sync_info=mybir.SyncInfo(on_wait=[w], on_update=[]),
                        bass_nofuse=True,
                    )
                    nc.register_instruction(nop, overwrite=True)
                    new.append(nop)
                inst.sync_info = mybir.SyncInfo(
                    on_wait=keep, on_update=list(si.on_update or []))
                changed = True
            new.append(inst)
        if changed:
            try:
                bb.instructions[:] = new
            except TypeError:
                bb.instructions.clear()
                for i_ in new:
                    bb.instructions.append(i_)


def fv(sl, dims):
    """Free-dim view: keep sl's partition dim, replace free dims (elem strides)."""
    return bass.AP(tensor=sl.tensor, offset=sl.offset, ap=[list(sl.ap[0])] + dims)


def build_nc():
    nc = bacc.Bacc("TRN2", target_bir_lowering=False, debug=False, num_devices=8)
    P = {}

    def par(name, shape, dt):
        P[name] = nc.declare_dram_parameter(name, shape, dt, isOutput=False)

    par("xb", [64, 128, 128], BF16)
    par("xband", [64, NPX], BF16)
    par("ones37", [1, NPX], BF16)
    par("waq", [128, 128], BF16)
    par("ba32", [32, 1], F32)
    par("wqk20", [1, 20], F32)
    par("sneg20", [1, 20], F32)
    par("svec", [1, 4], F32)
    par("wd2p", [21, 1], F32)
    par("wd2x512", [21, 1], F32)
    par("wcy", [81, 64], BF16)
    par("wf1t", [65, 16], BF16)
    par("wf2a", [49, 48], BF16)
    par("wf3t", [17, 64], BF16)
    out = nc.declare_dram_parameter("out", [64, 32 * 128], F32, isOutput=True)
    att_full = nc.declare_dram_parameter("att_full", [4, 133, 128], BF16,
                                         isOutput=True)

    with tile.TileContext(nc) as tc:
        with tc.tile_pool(name="main", bufs=1) as pool:
            _body(nc, tc, pool, P, out, att_full)

    nc.finalize()
    _split_multi_waits(nc)
    return nc


def _body(nc, tc, pool, P, out, att_full):
    AluOp = mybir.AluOpType

    # ---------- consts ----------
    def load(name, shape, dt):
        t = pool.tile(shape, dt, name=name, tag=name)
        nc.sync.dma_start(out=t[:, :], in_=P[name][:, :])
        return t

    waq_t = load("waq", [128, 128], BF16)
    ba32 = load("ba32", [32, 1], F32)
    wcy = load("wcy", [81, 64], BF16)
    wf1t = load("wf1t", [65, 16], BF16)
    wf2a = load("wf2a", [49, 48], BF16)
    wf3t = load("wf3t", [17, 64], BF16)
    wqk20 = load("wqk20", [1, 20], F32)
    sneg20 = load("sneg20", [1, 20], F32)
    svec = load("svec", [1, 4], F32)
    wd2p = load("wd2p", [21, 1], F32)
    wd2x512 = load("wd2x512", [21, 1], F32)

    # ---------- x quarters ----------
    xq = [pool.tile([128, 2048], BF16, name=f"xq{q}", tag=f"xq{q}")
          for q in range(4)]
    for q in range(4):
        src = bass.AP(tensor=P["xb"], offset=16 * q * 16384,
                      ap=[[2048, 8], [16384, 16], [1, 2048]])
        nc.sync.dma_start(out=xq[q][:, :], in_=src)

    # ---------- conv1 ----------
    a_bf = pool.tile([32, 2048], BF16)
    with tc.tile_pool(name="psa", bufs=1, space="PSUM") as psa:
        a_ps = psa.tile([32, 2048], F32)
        for n in range(4):
            for q in range(4):
                nc.tensor.matmul(
                    out=a_ps[:, n * 512:(n + 1) * 512],
                    lhsT=waq_t[:, q * 32:(q + 1) * 32],
                    rhs=xq[q][:, n * 512:(n + 1) * 512],
                    start=(q == 0), stop=(q == 3))
        nc.vector.tensor_scalar_add(out=a_bf[:, :], in0=a_ps[:, :],
                                    scalar1=ba32[:, :])

    # ---------- stats / LN scalars ----------
    stats4 = pool.tile([32, 4, 6], F32)
    for n in range(4):
        nc.vector.bn_stats(out=stats4[:, n, :], in_=a_bf[:, n * 512:(n + 1) * 512])
    statsT = pool.tile([1, 128, 6], F32)
    nc.sync.dma_start(out=statsT.rearrange("o p s -> o (p s)"),
                      in_=stats4.rearrange("p n s -> p (n s)"))
    mv = pool.tile([1, 2], F32)
    nc.vector.bn_aggr(out=mv[:, :], in_=statsT[:, :, :])
    sc = pool.tile([1, 8], F32)  # 0:mu 1:- 2:rinv 3..5 tmp
    r = 2.0 / 3.0
    nc.vector.tensor_scalar_mul(out=sc[:, 0:1], in0=mv[:, 0:1], scalar1=r)
    nc.vector.tensor_tensor(out=sc[:, 3:4], in0=mv[:, 0:1], in1=mv[:, 0:1],
                            op=AluOp.mult)
    nc.vector.tensor_scalar_mul(out=sc[:, 3:4], in0=sc[:, 3:4], scalar1=r)
    nc.vector.tensor_scalar_mul(out=sc[:, 4:5], in0=mv[:, 1:2], scalar1=r)
    nc.vector.tensor_add(out=sc[:, 3:4], in0=sc[:, 3:4], in1=sc[:, 4:5])
    nc.vector.tensor_tensor(out=sc[:, 4:5], in0=sc[:, 0:1], in1=sc[:, 0:1],
                            op=AluOp.mult)
    nc.vector.tensor_sub(out=sc[:, 3:4], in0=sc[:, 3:4], in1=sc[:, 4:5])
    nc.vector.tensor_scalar_add(out=sc[:, 3:4], in0=sc[:, 3:4], scalar1=1e-5)
    nc.scalar.sqrt(out=sc[:, 5:6], in_=sc[:, 3:4])
    nc.vector.reciprocal(out=sc[:, 2:3], in_=sc[:, 5:6])
    wqkrt = pool.tile([1, 20], F32)
    nc.vector.scalar_tensor_tensor(out=wqkrt[:, :], in0=sneg20[:, :],
                                   scalar=sc[:, 0:1], in1=wqk20[:, :],
                                   op0=AluOp.mult, op1=AluOp.add)
    wqkrt_bf = pool.tile([1, 20], BF16)
    nc.vector.tensor_copy(out=wqkrt_bf[:, :], in_=wqkrt[:, :])
    wqk_d = nc.dram_tensor("wqk_d", [1, 20], BF16)
    nc.sync.dma_start(out=wqk_d[:, :], in_=wqkrt_bf[:, :])
    wqkb = pool.tile([128, 20], BF16)
    nc.sync.dma_start(out=wqkb[:, :], in_=wqk_d[:, :].to_broadcast([128, 20]))
    aff = pool.tile([1, 2], F32)
    nc.vector.tensor_copy(out=aff[:, 0:1], in_=sc[:, 2:3])
    nc.vector.tensor_tensor(out=aff[:, 1:2], in0=sc[:, 0:1], in1=sc[:, 2:3],
                            op=AluOp.mult)
    aff_d = nc.dram_tensor("aff_d", [1, 2], F32)
    nc.sync.dma_start(out=aff_d[:, :], in_=aff[:, :])
    affb = pool.tile([128, 2], F32)
    nc.sync.dma_start(out=affb[:, :], in_=aff_d[:, :].to_broadcast([128, 2]))

    if STAGE < 2:
        nc.gpsimd.dma_start(out=out[:, :], in_=P['xband'][0:64, 0:4096])
        return
    # ---------- tokv gather ----------
    a_dram = nc.dram_tensor("a_dram", [32, 2048], BF16)
    # layout: [p0=(q4,c)][ (hr*32+gx)*16 + fy*4 + fx ]
    for fy in range(4):
        for hr in range(4):
            src = fv(a_bf[:, (4 * hr + fy) * 128:(4 * hr + fy) * 128 + 1],
                     [[1, 128]])
            dst = bass.AP(tensor=a_dram, offset=hr * 512 + fy * 4,
                          ap=[[2048, 32], [16, 32], [1, 4]])
            nc.sync.dma_start(out=dst, in_=src)
    tokv = pool.tile([128, 128 * 5], BF16)
    nc.vector.memset(fv(tokv[:, 4:5], [[5, 128], [1, 1]]), 1.0)
    for q4 in range(8):
        for c in range(4):
            ck0 = q4 * 16 + c * 4          # fy runs 0..3 from here
            p0 = q4 * 4 + c
            src = bass.AP(tensor=a_dram, offset=p0 * 2048,
                          ap=[[16, 128], [4, 4], [1, 4]])
            dst = fv(tokv[:, ck0 * 5:ck0 * 5 + 1], [[5, 4], [1, 4]])
            nc.sync.dma_start(out=dst, in_=src)

    # ---------- qk chain ----------
    tmp20 = pool.tile([128, 128 * 20], BF16)
    nc.vector.tensor_tensor(
        out=tmp20.rearrange("p (c k t) -> p c k t", c=128, k=4),
        in0=fv(tokv[:, 0:1], [[5, 128], [0, 4], [1, 5]]),
        in1=fv(wqkb[:, 0:1], [[0, 128], [5, 4], [1, 5]]),
        op=AluOp.mult)
    qkr = pool.tile([128, 128 * 4], F32)
    nc.vector.tensor_reduce(
        out=qkr.rearrange("p (c k) -> p c k", c=128),
        in_=tmp20.rearrange("p (c k t) -> p c k t", c=128, k=4),
        axis=mybir.AxisListType.X, op=AluOp.add)
    sq = pool.tile([128, 128 * 4], F32)
    nc.vector.tensor_tensor(out=sq[:, :], in0=qkr[:, :], in1=qkr[:, :],
                            op=AluOp.mult)
    ss = pool.tile([128, 128], F32)
    nc.vector.tensor_reduce(
        out=ss.rearrange("p (c o) -> p c o", c=128),
        in_=sq.rearrange("p (c k) -> p c k", c=128),
        axis=mybir.AxisListType.X, op=AluOp.add)
    nc.vector.tensor_scalar_add(out=ss[:, :], in0=ss[:, :], scalar1=1e-12)
    ssr = pool.tile([128, 128], F32)
    nc.scalar.sqrt(out=ssr[:, :], in_=ss[:, :])
    nc.vector.reciprocal(out=ssr[:, :], in_=ssr[:, :])
    Phi = pool.tile([128, 128 * 32], BF16)
    nc.vector.memset(fv(Phi[:, 0:1], [[32, 128], [1, 1]]), 1.0)
    nc.vector.memset(fv(Phi[:, 21:22], [[32, 128], [1, 11]]), 0.0)
    nc.vector.tensor_tensor(
        out=fv(Phi[:, 1:2], [[32, 128], [1, 4]]),
        in0=qkr.rearrange("p (c k) -> p c k", c=128),
        in1=fv(ssr[:, 0:1], [[1, 128], [0, 4]]),
        op=AluOp.mult)
    nc.vector.tensor_tensor(
        out=fv(Phi[:, 5:6], [[32, 128], [4, 4], [1, 4]]),
        in0=fv(Phi[:, 1:2], [[32, 128], [1, 4], [0, 4]]),
        in1=fv(Phi[:, 1:2], [[32, 128], [0, 4], [1, 4]]),
        op=AluOp.mult)

    if STAGE < 3:
        nc.gpsimd.dma_start(out=out[:, :], in_=P['xband'][0:64, 0:4096])
        return
    # ---------- A matmuls + post ----------
    A_sb = pool.tile([21, 80], F32)
    A_bf = pool.tile([21, 80], BF16)
    pp = pool.tile([1, 24], F32)
    ppT = pool.tile([21, 1], F32)
    ppT_bf = pool.tile([21, 1], BF16)
    pad512 = pool.tile([21, 1], F32)
    padv = pool.tile([1, 64], F32)
    padv_bf = pool.tile([1, 64], BF16)
    rec16 = pool.tile([1, 16], F32)
    with tc.tile_pool(name="psA", bufs=1, space="PSUM") as psA:
        A_ps = psA.tile([21, 80], F32)
        for g in range(16):
            for q4 in range(8):
                ck = q4 * 16 + g
                nc.tensor.matmul(
                    out=A_ps[:, g * 5:(g + 1) * 5],
                    lhsT=Phi[:, ck * 32:ck * 32 + 21],
                    rhs=tokv[:, ck * 5:(ck + 1) * 5],
                    start=(q4 == 0), stop=(q4 == 7))
        nc.vector.tensor_scalar_mul(out=A_sb[:, :], in0=A_ps[:, :],
                                    scalar1=wd2p[:, :])
    # pad-token feature vector: u = unit(-mu * s)
    nc.vector.tensor_scalar_mul(out=pp[:, 1:5], in0=svec[:, :], scalar1=sc[:, 0:1])
    nc.vector.tensor_scalar_mul(out=pp[:, 1:5], in0=pp[:, 1:5], scalar1=-1.0)
    sqp = pool.tile([1, 4], F32)
    nc.vector.tensor_tensor(out=sqp[:, :], in0=pp[:, 1:5], in1=pp[:, 1:5],
                            op=AluOp.mult)
    nc.vector.tensor_reduce(out=pp[:, 21:22], in_=sqp[:, :],
                            axis=mybir.AxisListType.X, op=AluOp.add)
    nc.vector.tensor_scalar_add(out=pp[:, 21:22], in0=pp[:, 21:22], scalar1=1e-12)
    nc.scalar.sqrt(out=pp[:, 22:23], in_=pp[:, 21:22])
    nc.vector.reciprocal(out=pp[:, 22:23], in_=pp[:, 22:23])
    nc.vector.tensor_scalar_mul(out=pp[:, 1:5], in0=pp[:, 1:5],
                                scalar1=pp[:, 22:23])
    nc.vector.memset(pp[:, 0:1], 1.0)
    nc.vector.tensor_tensor(
        out=fv(pp[:, 5:6], [[4, 4], [1, 4]]),
        in0=fv(pp[:, 1:2], [[1, 4], [0, 4]]),
        in1=fv(pp[:, 1:2], [[0, 4], [1, 4]]),
        op=AluOp.mult)
    nc.sync.dma_start(out=ppT[:, :], in_=pp[:, 0:21].rearrange("o d -> d o"))
    nc.vector.tensor_tensor(out=pad512[:, :], in0=ppT[:, :], in1=wd2x512[:, :],
                            op=AluOp.mult)
    nc.vector.tensor_tensor(
        out=fv(A_sb[:, 4:5], [[5, 16]]),
        in0=fv(A_sb[:, 4:5], [[5, 16]]),
        in1=fv(pad512[:, 0:1], [[0, 16]]),
        op=AluOp.add)
    nc.vector.tensor_copy(out=A_bf[:, :], in_=A_sb[:, :])
    nc.vector.tensor_copy(out=ppT_bf[:, :], in_=ppT[:, :])

    with tc.tile_pool(name="psP", bufs=1, space="PSUM") as psP:
        NUp = psP.tile([1, 80], F32)
        nc.tensor.matmul(out=NUp[:, :], lhsT=ppT_bf[:, :], rhs=A_bf[:, :],
                         start=True, stop=True)
        nc.vector.reciprocal(out=rec16[:, :], in_=fv(NUp[:, 4:5], [[5, 16]]))
        nc.vector.tensor_tensor(
            out=padv.rearrange("o (g j) -> o g j", g=16),
            in0=fv(NUp[:, 0:1], [[5, 16], [1, 4]]),
            in1=fv(rec16[:, 0:1], [[1, 16], [0, 4]]),
            op=AluOp.mult)
    nc.vector.tensor_scalar(out=padv[:, :], in0=padv[:, :],
                            scalar1=sc[:, 0:1], scalar2=sc[:, 2:3],
                            op0=AluOp.subtract, op1=AluOp.mult)
    nc.vector.tensor_copy(out=padv_bf[:, :], in_=padv[:, :])

    # pad rows -> att_full rows 0..68
    zr = pool.tile([4, 128], BF16)
    nc.vector.memset(zr[:, :], 0.0)
    nc.sync.dma_start(
        out=att_full[:, 0:5, :].rearrange("f r w -> f (r w)"),
        in_=fv(zr[:, 0:1], [[0, 5], [1, 128]]))
    padv_d = nc.dram_tensor("padv_d", [1, 64], BF16)
    nc.sync.dma_start(out=padv_d[:, :], in_=padv_bf[:, :])
    padrep = pool.tile([128, 64], BF16)
    nc.sync.dma_start(out=padrep[:, :], in_=padv_d[:, :].to_broadcast([128, 64]))
    # regroup cols (c,fy,fx) -> (c,fx,fy) so scatter runs are w-contiguous
    padrep4 = pool.tile([128, 64], BF16)
    nc.vector.tensor_copy(
        out=padrep4.rearrange("p (c fx fy) -> p c fx fy", c=4, fx=4),
        in_=fv(padrep[:, 0:1], [[16, 4], [1, 4], [4, 4]]))
    for c in range(4):
        for fx in range(4):
            base = fx * 17024 + (5 + c) * 128
            dst = bass.AP(tensor=att_full, offset=base,
                          ap=[[512, 4], [4, 32], [1, 4]])
            nc.sync.dma_start(
                out=dst, in_=padrep4[:, (c * 4 + fx) * 4:(c * 4 + fx) * 4 + 4])
    pp16 = pool.tile([4, 16 * 128], BF16)
    nc.sync.dma_start(out=pp16[:, :],
                      in_=att_full[:, 5:21, :].rearrange("f r w -> f (r w)"))
    for gq in range(1, 4):
        nc.sync.dma_start(
            out=att_full[:, 5 + 16 * gq:21 + 16 * gq, :]
                .rearrange("f r w -> f (r w)"),
            in_=pp16[:, :])

    if STAGE < 4:
        nc.gpsimd.dma_start(out=out[:, :], in_=P['xband'][0:64, 0:4096])
        return
    # ---------- NU ----------
    phiT = pool.tile([128, 16 * 128], BF16)
    for g in range(16):
        nc.sync.dma_start_transpose(out=phiT[:, g * 128:(g + 1) * 128],
                                    in_=Phi[:, g * 256:g * 256 + 128])
    if KSUB < 2:
        nc.gpsimd.dma_start(out=out[:, :], in_=P['xband'][0:64, 0:4096])
        return
    phiTq = [pool.tile([32, 16 * 128], BF16, name=f"phiTq{q}") for q in range(4)]
    for q4 in range(4):
        nc.sync.dma_start(out=phiTq[q4][:, :], in_=phiT[32 * q4:32 * q4 + 32, :])
    attv = pool.tile([128, 256], BF16)
    rec = pool.tile([128, 64], F32)
    with tc.tile_pool(name="psN", bufs=1, space="PSUM") as psN:
        NU_ps = psN.tile([128, 320], F32)
        for g in range(16):
            for q4 in range(4):
                u = g * 4 + q4
                c_, fy_ = g // 4, g % 4
                nc.tensor.matmul(
                    out=NU_ps[:, u * 5:(u + 1) * 5],
                    lhsT=phiTq[fy_][0:21, (q4 * 4 + c_) * 128:(q4 * 4 + c_ + 1) * 128],
                    rhs=A_bf[:, g * 5:(g + 1) * 5], start=True, stop=True)
        if KSUB < 3:
            dbg = pool.tile([128, 320], F32, name="dbg")
            nc.vector.tensor_copy(out=dbg[:, :], in_=NU_ps[:, :])
            nc.sync.dma_start(out=out[0:64, 0:320], in_=dbg[0:64, :])
            nc.gpsimd.dma_start(out=out[:, 320:4096], in_=P['xband'][0:64, 320:4096])
            return
        nc.vector.reciprocal(out=rec[:, :], in_=fv(NU_ps[:, 4:5], [[5, 64]]))
        nc.vector.tensor_scalar_mul(out=rec[:, :], in0=rec[:, :],
                                    scalar1=affb[:, 0:1])
        nc.vector.tensor_tensor(
            out=attv.rearrange("p (u j) -> p u j", u=64),
            in0=fv(NU_ps[:, 0:1], [[5, 64], [1, 4]]),
            in1=fv(rec[:, 0:1], [[1, 64], [0, 4]]),
            op=AluOp.mult)
    nc.vector.tensor_scalar_sub(out=attv[:, :], in0=attv[:, :],
                                scalar1=affb[:, 1:2])

    if KSUB < 4:
        nc.gpsimd.dma_start(out=out[:, :], in_=P['xband'][0:64, 0:4096])
        return
    # scatter real-query values: regroup attv cols (c,fy,q4,fx) -> (c,q4,fx,fy)
    attv4 = pool.tile([128, 256], BF16)
    nc.vector.tensor_copy(
        out=attv4.rearrange("p (c q fx fy) -> p c q fx fy", c=4, q=4, fx=4),
        in_=fv(attv[:, 0:1], [[64, 4], [4, 4], [1, 4], [16, 4]]))
    for c in range(4):
        for q4 in range(4):
            for fx in range(4):
                base = fx * 17024 + (69 + 16 * q4 + c) * 128
                dst = bass.AP(tensor=att_full, offset=base,
                              ap=[[512, 4], [4, 32], [1, 4]])
                blk = (c * 16 + q4 * 4 + fx) * 4
                nc.sync.dma_start(out=dst, in_=attv4[:, blk:blk + 4])

    if STAGE < 5:
        nc.gpsimd.dma_start(out=out[:, :], in_=P['xband'][0:64, 0:4096])
        return
    # ---------- y conv ----------
    rhsY = pool.tile([81, NPX], BF16)
    for jp in range(4):
        nc.sync.dma_start(
            out=rhsY[jp * 4:(jp + 1) * 4, :],
            in_=att_full[:, 32 * jp:32 * jp + 37, :].rearrange("f r w -> f (r w)"))
    nc.sync.dma_start(out=rhsY[16:17, :], in_=P["ones37"][:, :])
    nc.sync.dma_start(out=rhsY[17:81, :], in_=P["xband"][:, :])
    y_sb = pool.tile([65, NPX], BF16)
    nc.sync.dma_start(out=y_sb[64:65, :], in_=P["ones37"][:, :])
    with tc.tile_pool(name="psY", bufs=2, space="PSUM") as psY:
        for n in range(10):
            n0 = n * 512
            n1 = min(n0 + 512, NPX)
            y_ps = psY.tile([64, 512], F32, name=f"yps{n}", tag="yps")
            nc.tensor.matmul(out=y_ps[:, :n1 - n0], lhsT=wcy[:, :],
                             rhs=rhsY[:, n0:n1], start=True, stop=True)
            nc.vector.tensor_copy(out=y_sb[0:64, n0:n1], in_=y_ps[:, :n1 - n0])

    if STAGE < 6:
        nc.gpsimd.dma_start(out=out[:, :], in_=P['xband'][0:64, 0:4096])
        return
    # ---------- wf1 + relu -> h1c3 ----------
    h1c3 = pool.tile([49, NROW * PITCH], BF16)
    nc.vector.memset(fv(h1c3[0:48, 0:1], [[PITCH, NROW], [1, 2]]), 0.0)
    nc.vector.memset(fv(h1c3[0:48, 130:131], [[PITCH, NROW], [1, 2]]), 0.0)
    nc.vector.memset(h1c3[32:49, :], 1.0)  # rows 32..47 overwritten below
    with tc.tile_pool(name="psH1", bufs=2, space="PSUM") as psH1:
        for n in range(10):
            n0 = n * 512
            n1 = min(n0 + 512, NPX)
            h1_ps = psH1.tile([16, 512], F32, name=f"h1ps{n}", tag="h1ps")
            nc.tensor.matmul(out=h1_ps[:, :n1 - n0], lhsT=wf1t[:, :],
                             rhs=y_sb[:, n0:n1], start=True, stop=True)
            nr = (n1 - n0) // 128
            dst = fv(h1c3[0:16, (n0 // 128) * PITCH + 2:(n0 // 128) * PITCH + 3],
                     [[PITCH, nr], [1, 128]])
            nc.vector.tensor_relu(
                out=dst, in_=h1_ps.rearrange("p (r w) -> p r w", w=128)[:, :nr, :])
    L1 = NROW * PITCH
    nc.sync.dma_start(out=fv(h1c3[16:32, 0:1], [[1, L1 - 2]]),
                      in_=fv(h1c3[0:16, 2:3], [[1, L1 - 2]]))
    nc.sync.dma_start(out=fv(h1c3[32:48, 2:3], [[1, L1 - 2]]),
                      in_=fv(h1c3[0:16, 0:1], [[1, L1 - 2]]))

    # ---------- h2 ----------
    h2r = pool.tile([17, 32 * 128], BF16)
    nc.vector.memset(h2r[:, :], 1.0)  # row 16 stays ones; rows 0..15 overwritten
    with tc.tile_pool(name="psH2", bufs=2, space="PSUM") as psH2:
        for n in range(8):
            h2_ps = psH2.tile([16, 512], F32, name=f"h2ps{n}", tag="h2ps")
            for ky in range(3):
                base = (2 * ky + 4 * n) * PITCH + 2
                rhs = fv(h1c3[0:49, base:base + 1], [[PITCH, 4], [1, 128]])
                nc.tensor.matmul(out=h2_ps[:, :], lhsT=wf2a[:, ky * 16:(ky + 1) * 16],
                                 rhs=rhs, start=(ky == 0), stop=(ky == 2))
            nc.vector.tensor_relu(out=h2r[0:16, n * 512:(n + 1) * 512],
                                  in_=h2_ps[:, :])

    # ---------- wf3 + residual ----------
    out_t = pool.tile([64, 32 * 128], F32)
    with tc.tile_pool(name="psO", bufs=2, space="PSUM") as psO:
        for n in range(8):
            o_ps = psO.tile([64, 512], F32, name=f"ops{n}", tag="ops")
            nc.tensor.matmul(out=o_ps[:, :], lhsT=wf3t[:, :],
                             rhs=h2r[:, n * 512:(n + 1) * 512],
                             start=True, stop=True)
            nc.vector.tensor_add(out=out_t[:, n * 512:(n + 1) * 512],
                                 in0=o_ps[:, :],
                                 in1=y_sb[0:64, 640 + n * 512:640 + (n + 1) * 512])
    nc.sync.dma_start(out=out[:, :], in_=out_t[:, :])


def _host_prep(inputs):
    x = np.asarray(inputs["x"], np.float32)
    g32 = lambda k: np.asarray(inputs[k], np.float32)
    w1a, b1a, w1b, b1b = g32("w1a"), g32("b1a"), g32("w1b"), g32("b1b")
    wqk = g32("wqk")
    w2, b2, wc, bc = g32("w2"), g32("b2"), g32("wc"), g32("bc")
    wf1, bf1, wf2, bf2, wf3, bf3 = (g32("wf1"), g32("bf1"), g32("wf2"),
                                    g32("bf2"), g32("wf3"), g32("bf3"))

    Wa = (w1b @ w1a).astype(np.float32)
    ba = (w1b @ b1a + b1b).astype(np.float32)
    Wc2 = (wc @ w2).astype(np.float32)
    bc2 = (wc @ b2 + bc).astype(np.float32)
    s_col = wqk.sum(axis=0).astype(np.float32)

    waq = np.zeros((128, 128), np.float32)
    for q in range(4):
        for hb in range(8):
            waq[hb * 16:(hb + 1) * 16, q * 32 + hb * 4:q * 32 + hb * 4 + 4] = \
                Wa[:, 16 * q:16 * q + 16].T
    ba32 = np.tile(ba, 8).reshape(32, 1).astype(np.float32)

    wqk20 = np.zeros((1, 20), np.float32)
    sneg20 = np.zeros((1, 20), np.float32)
    for k in range(4):
        wqk20[0, k * 5:k * 5 + 4] = wqk[:, k]
        sneg20[0, k * 5 + 4] = -s_col[k]
    svec = s_col.reshape(1, 4).astype(np.float32)

    wd2 = np.concatenate([[1.0], np.full(4, 1.0 / S),
                          np.full(16, 1.0 / (2.0 * S * S))]).astype(np.float32)

    wf1t = np.zeros((65, 16), np.float32)
    wf1t[:64] = wf1.T
    wf1t[64] = bf1
    # h1c3 blocks: r=0 center (kx=1), r=1 holds h1[w+2] (kx=2), r=2 h1[w-2] (kx=0)
    wf2a = np.zeros((49, 48), np.float32)
    for ky in range(3):
        for kx in range(3):
            blk = (kx + 2) % 3
            wf2a[blk * 16:(blk + 1) * 16, ky * 16:(ky + 1) * 16] = \
                wf2[:, :, ky, kx].T
    wf2a[48, 2 * 16:3 * 16] = bf2
    wf3t = np.zeros((17, 64), np.float32)
    wf3t[:16] = wf3.T
    wf3t[16] = bf3

    common = dict(
        waq=waq.astype(BF), ba32=ba32,
        wqk20=wqk20, sneg20=sneg20, svec=svec,
        wd2p=wd2.reshape(21, 1), wd2x512=(512.0 * wd2).reshape(21, 1),
        wf1t=wf1t.astype(BF), wf2a=wf2a.astype(BF), wf3t=wf3t.astype(BF))

    in_maps = []
    for i in range(8):
        b, j = i // 4, i % 4
        m = dict(common)
        m["xb"] = np.ascontiguousarray(x[b]).astype(BF)
        xband = np.zeros((64, 37, 128), np.float32)
        ones37 = np.ones((1, 37, 128), np.float32)
        if j == 0:
            xband[:, 5:] = x[b][:, 0:32]
            ones37[:, :5] = 0.0
        else:
            xband[:] = x[b][:, 32 * j - 5:32 * j + 32]
        m["xband"] = xband.reshape(64, NPX).astype(BF)
        m["ones37"] = ones37.reshape(1, NPX).astype(BF)
        wcy = np.zeros((81, 64), np.float32)
        wcy[4 * j:4 * j + 4, :] = Wc2.T
        wcy[16, :] = bc2
        wcy[17:81, :] = np.eye(64, dtype=np.float32)
        m["wcy"] = wcy.astype(BF)
        in_maps.append(m)
    return in_maps


def kernel(**inputs):
    if "nc" not in _CACHED:
        _CACHED["nc"] = build_nc()
    nc = _CACHED["nc"]
    in_maps = _host_prep(inputs)
    res = run_bass_kernel_spmd(nc, in_maps, list(range(8)))
    _CACHED["last_result"] = res
    out = np.zeros((B, C, H, W), np.float32)
    for i in range(8):
        b, j = i // 4, i % 4
        out[b, :, 32 * j:32 * j + 32, :] = \
            res.results[i]["out"].reshape(64, 32, 128)
    return out



# revision 22
# speedup vs baseline: 1.3004x; 1.3004x over previous
"""GridAttention TRN2 kernel: 8-core SPMD Bass implementation (v2).

Sharding: core i handles batch b=i//4, output row band j=i%4 (rows 32j..32j+32).
Each core reads its batch's full x (bf16), computes conv1 -> linearized grid
attention (2nd-order softmax Taylor feature map) -> residual conv stack -> FF,
and writes its 32-row f32 output band.

v2 data path: conv1 writes a (fy,fx)-major permuted layout so 16 XBAR DMA
transposes build the token tile in SBUF (no DRAM gather round-trip); A and NU
use block-diagonal batched matmuls (6 groups per call); Phi^T comes from PE
transposes; pad rows are written with 2 large DMAs; DMAs alternate between the
two HWDGE engines (sync/scalar); the output store is split per block and
overlapped with the FF tail.
"""
import numpy as np
import ml_dtypes
import os
STAGE = int(os.environ.get("KSTAGE", "9"))
KDUMP = int(os.environ.get("KDUMP", "0"))

import concourse.bass as bass
import concourse.bacc as bacc
import concourse.tile as tile
from concourse import mybir
from concourse.bass_utils import run_bass_kernel_spmd

B, C, H, W = 2, 64, 128, 128
S = np.float32(32.0 ** 0.5)
NROW = 37
NPX = NROW * 128
PITCH = 132
F32 = mybir.dt.float32
BF16 = mybir.dt.bfloat16
BF = ml_dtypes.bfloat16
GB = [(0, 6), (6, 6), (12, 4)]          # group batches for block-diag matmuls

_CACHED = {}


def _split_multi_waits(nc, max_waits=1):
    """This neuronxcc accepts only ONE sync-wait per instruction; Tile emits
    several.  Move extras onto InstNoOps inserted just before, same engine."""
    f = nc.m.functions[0]
    for bb in f.blocks:
        insts = list(bb.instructions)
        new, changed = [], False
        for inst in insts:
            si = getattr(inst, "sync_info", None)
            if si is not None and si.on_wait and len(si.on_wait) > max_waits:
                waits = list(si.on_wait)
                extra, keep = waits[:-max_waits], waits[-max_waits:]
                for w in extra:
                    nop = mybir.InstNoOp(
                        name=nc.get_next_instruction_name(),
                        engine=inst.engine,
                        sync_info=mybir.SyncInfo(on_wait=[w], on_update=[]),
                        bass_nofuse=True,
                    )
                    nc.register_instruction(nop, overwrite=True)
                    new.append(nop)
                inst.sync_info = mybir.SyncInfo(
                    on_wait=keep, on_update=list(si.on_update or []))
                changed = True
            new.append(inst)
        if changed:
            try:
                bb.instructions[:] = new
            except TypeError:
                bb.instructions.clear()
                for i_ in new:
                    bb.instructions.append(i_)


def fv(sl, dims):
    """Free-dim view: keep sl's partition dim, replace free dims (elem strides)."""
    return bass.AP(tensor=sl.tensor, offset=sl.offset, ap=[list(sl.ap[0])] + dims)


def build_nc():
    nc = bacc.Bacc("TRN2", target_bir_lowering=False, debug=False, num_devices=8)
    P = {}

    def par(name, shape, dt):
        P[name] = nc.declare_dram_parameter(name, shape, dt, isOutput=False)

    par("xb", [64, 128, 128], BF16)
    par("xband", [64, NPX], BF16)
    par("ones37", [1, NPX], BF16)
    par("waq", [128, 128], BF16)
    par("ba32", [32, 1], F32)
    par("wqk20", [1, 20], F32)
    par("sneg20", [1, 20], F32)
    par("svec", [1, 4], F32)
    par("wd2p126", [126, 1], F32)
    par("pad_rhs", [1, 96], F32)
    par("diagmask", [126, 96], BF16)
    par("ident", [128, 128], BF16)
    par("wcy", [81, 64], BF16)
    par("wf1t", [65, 16], BF16)
    par("wf2a", [49, 48], BF16)
    par("wf3t", [17, 64], BF16)
    out = nc.declare_dram_parameter("out", [64, 32 * 128], F32, isOutput=True)
    att_full = nc.declare_dram_parameter("att_full", [4, 133, 128], BF16,
                                         isOutput=True)

    with tile.TileContext(nc) as tc:
        with tc.tile_pool(name="main", bufs=1) as pool:
            _body(nc, tc, pool, P, out, att_full)

    nc.finalize()
    _split_multi_waits(nc)
    return nc


def _body(nc, tc, pool, P, out, att_full):
    AluOp = mybir.AluOpType
    ENG = [nc.sync, nc.scalar]

    # ---------- consts ----------
    def load(name, shape, dt, ei=1):
        t = pool.tile(shape, dt, name=name, tag=name)
        ENG[ei].dma_start(out=t[:, :], in_=P[name][:, :])
        return t

    waq_t = load("waq", [128, 128], BF16, 0)
    ba32 = load("ba32", [32, 1], F32)
    wcy = load("wcy", [81, 64], BF16)
    wf1t = load("wf1t", [65, 16], BF16)
    wf2a = load("wf2a", [49, 48], BF16)
    wf3t = load("wf3t", [17, 64], BF16)
    wqk20 = load("wqk20", [1, 20], F32)
    sneg20 = load("sneg20", [1, 20], F32)
    svec = load("svec", [1, 4], F32)
    wd2p126 = load("wd2p126", [126, 1], F32)
    pad_rhs = load("pad_rhs", [1, 96], F32)
    diagmask = load("diagmask", [126, 96], BF16, 0)
    ident = load("ident", [128, 128], BF16, 0)

    # ---------- x quarters (alternate engines for parallel queues) ----------
    xq = [pool.tile([128, 2048], BF16, name=f"xq{q}", tag=f"xq{q}")
          for q in range(4)]
    for q in range(4):
        src = bass.AP(tensor=P["xb"], offset=16 * q * 16384,
                      ap=[[2048, 8], [16384, 16], [1, 2048]])
        ENG[q % 2].dma_start(out=xq[q][:, :], in_=src)

    # ---------- conv1 -> a_perm [(hb,oc)=32, (fy,fx,hr,gx)=2048] ----------
    a_perm = pool.tile([32, 2048], BF16)
    with tc.tile_pool(name="psa", bufs=1, space="PSUM") as psa:
        a_ps = psa.tile([32, 2048], F32)
        for n in range(4):
            for q in range(4):
                nc.tensor.matmul(
                    out=a_ps[:, n * 512:(n + 1) * 512],
                    lhsT=waq_t[:, q * 32:(q + 1) * 32],
                    rhs=xq[q][:, n * 512:(n + 1) * 512],
                    start=(q == 0), stop=(q == 3))
            # PSUM block n covers rows r16 in [4n,4n+4): (fy,gx,fx) order.
            # Write permuted: elem offset fy*512 + fx*128 + hr*32 + gx, hr=n.
            nc.vector.tensor_scalar_add(
                out=fv(a_perm[:, n * 32:n * 32 + 1],
                       [[512, 4], [1, 32], [128, 4]]),
                in0=fv(a_ps[:, n * 512:n * 512 + 1],
                       [[128, 4], [4, 32], [1, 4]]),
                scalar1=ba32[:, :])

    # ---------- stats / LN scalars ----------
    stats4 = pool.tile([32, 4, 6], F32)
    for n in range(4):
        nc.vector.bn_stats(out=stats4[:, n, :],
                           in_=a_perm[:, n * 512:(n + 1) * 512])
    statsT = pool.tile([1, 128, 6], F32)
    nc.sync.dma_start(out=statsT.rearrange("o p s -> o (p s)"),
                      in_=stats4.rearrange("p n s -> p (n s)"))
    mv = pool.tile([1, 2], F32)
    nc.vector.bn_aggr(out=mv[:, :], in_=statsT[:, :, :])
    sc = pool.tile([1, 8], F32)  # 0:mu 1:- 2:rinv 3..5 tmp
    r = 2.0 / 3.0
    nc.vector.tensor_scalar_mul(out=sc[:, 0:1], in0=mv[:, 0:1], scalar1=r)
    nc.vector.tensor_tensor(out=sc[:, 3:4], in0=mv[:, 0:1], in1=mv[:, 0:1],
                            op=AluOp.mult)
    nc.vector.tensor_scalar_mul(out=sc[:, 3:4], in0=sc[:, 3:4], scalar1=r)
    nc.vector.tensor_scalar_mul(out=sc[:, 4:5], in0=mv[:, 1:2], scalar1=r)
    nc.vector.tensor_add(out=sc[:, 3:4], in0=sc[:, 3:4], in1=sc[:, 4:5])
    nc.vector.tensor_tensor(out=sc[:, 4:5], in0=sc[:, 0:1], in1=sc[:, 0:1],
                            op=AluOp.mult)
    nc.vector.tensor_sub(out=sc[:, 3:4], in0=sc[:, 3:4], in1=sc[:, 4:5])
    nc.vector.tensor_scalar_add(out=sc[:, 3:4], in0=sc[:, 3:4], scalar1=1e-5)
    nc.scalar.sqrt(out=sc[:, 5:6], in_=sc[:, 3:4])
    nc.vector.reciprocal(out=sc[:, 2:3], in_=sc[:, 5:6])
    wqkrt = pool.tile([1, 20], F32)
    nc.vector.scalar_tensor_tensor(out=wqkrt[:, :], in0=sneg20[:, :],
                                   scalar=sc[:, 0:1], in1=wqk20[:, :],
                                   op0=AluOp.mult, op1=AluOp.add)
    wqkrt_bf = pool.tile([1, 20], BF16)
    nc.vector.tensor_copy(out=wqkrt_bf[:, :], in_=wqkrt[:, :])
    wqk_d = nc.dram_tensor("wqk_d", [1, 20], BF16)
    nc.sync.dma_start(out=wqk_d[:, :], in_=wqkrt_bf[:, :])
    wqkb = pool.tile([128, 20], BF16)
    nc.sync.dma_start(out=wqkb[:, :], in_=wqk_d[:, :].to_broadcast([128, 20]))
    aff = pool.tile([1, 2], F32)
    nc.vector.tensor_copy(out=aff[:, 0:1], in_=sc[:, 2:3])
    nc.vector.tensor_tensor(out=aff[:, 1:2], in0=sc[:, 0:1], in1=sc[:, 2:3],
                            op=AluOp.mult)
    aff_d = nc.dram_tensor("aff_d", [1, 2], F32)
    nc.scalar.dma_start(out=aff_d[:, :], in_=aff[:, :])
    affb = pool.tile([128, 2], F32)
    nc.scalar.dma_start(out=affb[:, :], in_=aff_d[:, :].to_broadcast([128, 2]))

    # ---------- pad-token feature vector u = unit(-mu * s) ----------
    pp = pool.tile([1, 24], F32)
    nc.vector.tensor_scalar_mul(out=pp[:, 1:5], in0=svec[:, :], scalar1=sc[:, 0:1])
    nc.vector.tensor_scalar_mul(out=pp[:, 1:5], in0=pp[:, 1:5], scalar1=-1.0)
    sqp = pool.tile([1, 4], F32)
    nc.vector.tensor_tensor(out=sqp[:, :], in0=pp[:, 1:5], in1=pp[:, 1:5],
                            op=AluOp.mult)
    nc.vector.tensor_reduce(out=pp[:, 21:22], in_=sqp[:, :],
                            axis=mybir.AxisListType.X, op=AluOp.add)
    nc.vector.tensor_scalar_add(out=pp[:, 21:22], in0=pp[:, 21:22], scalar1=1e-12)
    nc.scalar.sqrt(out=pp[:, 22:23], in_=pp[:, 21:22])
    nc.vector.reciprocal(out=pp[:, 22:23], in_=pp[:, 22:23])
    nc.vector.tensor_scalar_mul(out=pp[:, 1:5], in0=pp[:, 1:5],
                                scalar1=pp[:, 22:23])
    nc.vector.memset(pp[:, 0:1], 1.0)
    nc.vector.tensor_tensor(
        out=fv(pp[:, 5:6], [[4, 4], [1, 4]]),
        in0=fv(pp[:, 1:2], [[1, 4], [0, 4]]),
        in1=fv(pp[:, 1:2], [[0, 4], [1, 4]]),
        op=AluOp.mult)
    # pp tiled 6x along free (f32, stationary of the pad A-matmul)
    pp126T = pool.tile([1, 126], F32)
    nc.vector.tensor_copy(out=pp126T[:, :], in_=fv(pp[:, 0:1], [[0, 6], [1, 21]]))
    pp126T_bf = pool.tile([1, 126], BF16)
    nc.vector.tensor_copy(out=pp126T_bf[:, :], in_=pp126T[:, :])
    ppd = nc.dram_tensor("ppd", [1, 126], BF16)
    nc.scalar.dma_start(out=ppd[:, :], in_=pp126T_bf[:, :])
    ppT126 = pool.tile([126, 1], BF16)
    nc.scalar.dma_start(
        out=ppT126[:, :],
        in_=bass.AP(tensor=ppd, offset=0, ap=[[1, 126], [1, 1]]))

    if KDUMP == 8:      # dump pp [1,24], pp126T [1,126], ppT126 [126,1]
        nc.gpsimd.dma_start(
            out=bass.AP(tensor=out, offset=0, ap=[[1, 1], [1, 24]]),
            in_=pp[:, :])
        nc.gpsimd.dma_start(
            out=bass.AP(tensor=out, offset=32, ap=[[1, 1], [1, 126]]),
            in_=pp126T[:, :])
        nc.gpsimd.dma_start(
            out=bass.AP(tensor=out, offset=160, ap=[[1, 126], [1, 1]]),
            in_=ppT126[:, :])
        return
    if STAGE < 2:
        nc.gpsimd.dma_start(out=out[:, :], in_=P['xband'][0:64, 0:4096])
        return

    # ---------- token tile via XBAR transposes ----------
    # tokT[p=(hr,gx), (fy*4+fx)*32 + (q4*4+oc)] = a[oc, r=16q4+4hr+fy, w=4gx+fx]
    tokT = pool.tile([128, 512], BF16)
    for u in range(16):
        ENG[u % 2].dma_start_transpose(
            out=tokT[:, u * 32:(u + 1) * 32],
            in_=a_perm[:, u * 128:(u + 1) * 128])
    # tokv[p, ck*5 + fx], ck = q4*16 + c*4 + fy  (5th slot = ones)
    tokv = pool.tile([128, 128 * 5], BF16)
    nc.vector.memset(fv(tokv[:, 4:5], [[5, 128], [1, 1]]), 1.0)
    for fy in range(4):
        nc.vector.tensor_copy(
            out=fv(tokv[:, fy * 5:fy * 5 + 1], [[80, 8], [20, 4], [1, 4]]),
            in_=fv(tokT[:, fy * 128:fy * 128 + 1], [[4, 8], [1, 4], [32, 4]]))

    if KDUMP == 1:      # dump tokv [128, 640]
        nc.gpsimd.dma_start(
            out=bass.AP(tensor=out, offset=0, ap=[[640, 128], [1, 640]]),
            in_=tokv[:, :])
        return
    # ---------- qk chain ----------
    tmp20 = pool.tile([128, 128 * 20], BF16)
    nc.vector.tensor_tensor(
        out=tmp20.rearrange("p (c k t) -> p c k t", c=128, k=4),
        in0=fv(tokv[:, 0:1], [[5, 128], [0, 4], [1, 5]]),
        in1=fv(wqkb[:, 0:1], [[0, 128], [5, 4], [1, 5]]),
        op=AluOp.mult)
    qkr = pool.tile([128, 128 * 4], F32)
    nc.vector.tensor_reduce(
        out=qkr.rearrange("p (c k) -> p c k", c=128),
        in_=tmp20.rearrange("p (c k t) -> p c k t", c=128, k=4),
        axis=mybir.AxisListType.X, op=AluOp.add)
    sq = pool.tile([128, 128 * 4], F32)
    nc.vector.tensor_tensor(out=sq[:, :], in0=qkr[:, :], in1=qkr[:, :],
                            op=AluOp.mult)
    ss = pool.tile([128, 128], F32)
    nc.vector.tensor_reduce(
        out=ss.rearrange("p (c o) -> p c o", c=128),
        in_=sq.rearrange("p (c k) -> p c k", c=128),
        axis=mybir.AxisListType.X, op=AluOp.add)
    nc.vector.tensor_scalar_add(out=ss[:, :], in0=ss[:, :], scalar1=1e-12)
    ssr = pool.tile([128, 128], F32)
    nc.scalar.sqrt(out=ssr[:, :], in_=ss[:, :])
    nc.vector.reciprocal(out=ssr[:, :], in_=ssr[:, :])
    # Phi [128, ck*21 + k]: 1, q(4), outer(16)
    Phi = pool.tile([128, 128 * 21], BF16)
    nc.vector.memset(fv(Phi[:, 0:1], [[21, 128], [1, 1]]), 1.0)
    nc.vector.tensor_tensor(
        out=fv(Phi[:, 1:2], [[21, 128], [1, 4]]),
        in0=qkr.rearrange("p (c k) -> p c k", c=128),
        in1=fv(ssr[:, 0:1], [[1, 128], [0, 4]]),
        op=AluOp.mult)
    nc.vector.tensor_tensor(
        out=fv(Phi[:, 5:6], [[21, 128], [4, 4], [1, 4]]),
        in0=fv(Phi[:, 1:2], [[21, 128], [1, 4], [0, 4]]),
        in1=fv(Phi[:, 1:2], [[21, 128], [0, 4], [1, 4]]),
        op=AluOp.mult)

    if KDUMP == 2:      # dump Phi first 2048 cols
        nc.gpsimd.dma_start(
            out=bass.AP(tensor=out, offset=0, ap=[[2048, 128], [1, 2048]]),
            in_=Phi[:, 0:2048])
        return
    if STAGE < 3:
        nc.gpsimd.dma_start(out=out[:, :], in_=P['xband'][0:64, 0:4096])
        return

    # ---------- A matmuls (block-diag batched) + pad contribution ----------
    # A126sb[(gi,k), bi*30 + gi*5 + j] = wd2[k] * A_g[k, j] (zeros off-diag)
    A126sb = pool.tile([126, 96], BF16)
    with tc.tile_pool(name="psA", bufs=1, space="PSUM") as psA:
        A_ps = psA.tile([126, 96], F32)
        nc.vector.memset(A_ps[:, :], 0.0)  # regions no matmul touches
        for bi, (g0, gsz) in enumerate(GB):
            for q4 in range(8):
                nc.tensor.matmul(
                    out=A_ps[0:gsz * 21, bi * 30:bi * 30 + gsz * 5],
                    lhsT=Phi[:, (q4 * 16 + g0) * 21:(q4 * 16 + g0 + gsz) * 21],
                    rhs=tokv[:, (q4 * 16 + g0) * 5:(q4 * 16 + g0 + gsz) * 5],
                    start=(q4 == 0), stop=False)
            nc.tensor.matmul(
                out=A_ps[0:gsz * 21, bi * 30:bi * 30 + gsz * 5],
                lhsT=pp126T[:, 0:gsz * 21],
                rhs=pad_rhs[:, bi * 30:bi * 30 + gsz * 5],
                start=False, stop=True)
        # scale all features by wd2, zero the off-diagonal blocks via mask
        A126f = pool.tile([126, 96], F32)
        nc.vector.tensor_scalar_mul(out=A126f[:, :], in0=A_ps[:, :],
                                    scalar1=wd2p126[:, :])
        nc.vector.tensor_tensor(out=A126sb[:, :], in0=A126f[:, :],
                                in1=diagmask[:, :], op=AluOp.mult)

    if KDUMP == 3:      # dump A126sb [126, 96]
        nc.gpsimd.dma_start(
            out=bass.AP(tensor=out, offset=0, ap=[[96, 126], [1, 96]]),
            in_=A126sb[:, :])
        return
    # ---------- pad-query outputs ----------
    padv = pool.tile([1, 64], F32)
    padv_bf = pool.tile([1, 64], BF16)
    rec16 = pool.tile([1, 16], F32)
    with tc.tile_pool(name="psP", bufs=1, space="PSUM") as psP:
        NUp = psP.tile([1, 96], F32)
        nc.tensor.matmul(out=NUp[:, :], lhsT=ppT126[:, :], rhs=A126sb[:, :],
                         start=True, stop=True)
        nc.vector.reciprocal(out=rec16[:, :], in_=fv(NUp[:, 4:5], [[5, 16]]))
        nc.vector.tensor_tensor(
            out=padv.rearrange("o (g j) -> o g j", g=16),
            in0=fv(NUp[:, 0:1], [[5, 16], [1, 4]]),
            in1=fv(rec16[:, 0:1], [[1, 16], [0, 4]]),
            op=AluOp.mult)
    nc.vector.tensor_scalar(out=padv[:, :], in0=padv[:, :],
                            scalar1=sc[:, 0:1], scalar2=sc[:, 2:3],
                            op0=AluOp.subtract, op1=AluOp.mult)
    nc.vector.tensor_copy(out=padv_bf[:, :], in_=padv[:, :])

    if KDUMP == 5:      # dump padv [1, 64]
        nc.gpsimd.dma_start(
            out=bass.AP(tensor=out, offset=0, ap=[[64, 1], [1, 64]]),
            in_=padv[:, :])
        return
    # pad rows -> att_full rows 0..69 with 3 big DMAs
    zr = pool.tile([4, 128], BF16)
    nc.vector.memset(zr[:, :], 0.0)
    nc.sync.dma_start(
        out=att_full[:, 0:5, :].rearrange("f r w -> f (r w)"),
        in_=fv(zr[:, 0:1], [[0, 5], [1, 128]]))
    # padv [1,(g,fx)] -> DRAM -> padsm [4=fx, 16=g]
    padv_d = nc.dram_tensor("padv_d", [1, 64], BF16)
    nc.scalar.dma_start(out=padv_d[:, :], in_=padv_bf[:, :])
    padsm = pool.tile([4, 16], BF16)
    nc.scalar.dma_start(
        out=padsm[:, :],
        in_=bass.AP(tensor=padv_d, offset=0, ap=[[1, 4], [4, 16]]))
    # padfull[fx, rr*128 + gx*4 + fy] = padsm[fx, rr*4 + fy]  (c = att_row%4 = rr)
    padfull = pool.tile([4, 512], BF16)
    for rr in range(4):
        nc.vector.tensor_copy(
            out=fv(padfull[:, rr * 128:rr * 128 + 1], [[4, 32], [1, 4]]),
            in_=fv(padsm[:, rr * 4:rr * 4 + 1], [[0, 32], [1, 4]]))
    nc.sync.dma_start(
        out=att_full[:, 5:69, :].rearrange("f r w -> f (r w)"),
        in_=fv(padfull[:, 0:1], [[0, 16], [1, 512]]))

    if STAGE < 4:
        nc.gpsimd.dma_start(out=out[:, :], in_=P['xband'][0:64, 0:4096])
        return

    # ---------- NU for chunks 0..3 (att rows 64..128) ----------
    # attv[p, q*64 + g*4 + fx]
    attv = pool.tile([128, 256], BF16)
    with tc.tile_pool(name="psT", bufs=2, space="PSUM") as psT, \
         tc.tile_pool(name="psN", bufs=2, space="PSUM") as psN, \
         tc.tile_pool(name="sbT", bufs=2) as sbT:
        for q4 in range(4):
            NU_ps = psN.tile([128, 96], F32, name=f"nups{q4}", tag="nups")
            for bi, (g0, gsz) in enumerate(GB):
                phT_ps = psT.tile([126, 128], BF16, name=f"pht{q4}_{bi}",
                                  tag="pht")
                nc.tensor.transpose(
                    out=phT_ps[0:gsz * 21, :],
                    in_=Phi[:, (q4 * 16 + g0) * 21:(q4 * 16 + g0 + gsz) * 21],
                    identity=ident[:, :])
                phT_sb = sbT.tile([126, 128], BF16, name=f"phs{q4}_{bi}",
                                  tag="phs")
                nc.vector.tensor_copy(out=phT_sb[0:gsz * 21, :],
                                      in_=phT_ps[0:gsz * 21, :])
                if KDUMP == 6 and q4 == 0 and bi == 0:
                    nc.gpsimd.dma_start(
                        out=bass.AP(tensor=out, offset=0,
                                    ap=[[128, 126], [1, 128]]),
                        in_=phT_sb[0:126, :])
                    return
                nc.tensor.matmul(
                    out=NU_ps[:, bi * 30:bi * 30 + gsz * 5],
                    lhsT=phT_sb[0:gsz * 21, :],
                    rhs=A126sb[0:gsz * 21, bi * 30:bi * 30 + gsz * 5],
                    start=True, stop=True)
            if KDUMP == 7 and q4 == 0:
                nudbg = pool.tile([128, 96], F32, name="nudbg")
                nc.vector.tensor_copy(out=nudbg[:, :], in_=NU_ps[:, :])
                nc.gpsimd.dma_start(
                    out=bass.AP(tensor=out, offset=0, ap=[[96, 128], [1, 96]]),
                    in_=nudbg[:, :])
                return
            rec = pool.tile([128, 16], F32, name=f"rec{q4}", tag=f"rec{q4}")
            nc.vector.reciprocal(out=rec[:, :], in_=fv(NU_ps[:, 4:5], [[5, 16]]))
            nc.vector.tensor_scalar_mul(out=rec[:, :], in0=rec[:, :],
                                        scalar1=affb[:, 0:1])
            nc.vector.tensor_tensor(
                out=fv(attv[:, q4 * 64:q4 * 64 + 1], [[4, 16], [1, 4]]),
                in0=fv(NU_ps[:, 0:1], [[5, 16], [1, 4]]),
                in1=fv(rec[:, 0:1], [[1, 16], [0, 4]]),
                op=AluOp.mult)
    nc.vector.tensor_scalar_sub(out=attv[:, :], in0=attv[:, :],
                                scalar1=affb[:, 1:2])

    if KDUMP == 4:      # dump attv [128, 256]
        nc.gpsimd.dma_start(
            out=bass.AP(tensor=out, offset=0, ap=[[256, 128], [1, 256]]),
            in_=attv[:, :])
        return
    # scatter real-query values: regroup attv cols (q,c,fy,fx) -> (c,q,fx,fy)
    attv4 = pool.tile([128, 256], BF16)
    nc.vector.tensor_copy(
        out=attv4.rearrange("p (c q fx fy) -> p c q fx fy", c=4, q=4, fx=4),
        in_=fv(attv[:, 0:1], [[16, 4], [64, 4], [1, 4], [4, 4]]))
    k = 0
    for c in range(4):
        for q4 in range(4):
            for fx in range(4):
                base = fx * 17024 + (69 + 16 * q4 + c) * 128
                dst = bass.AP(tensor=att_full, offset=base,
                              ap=[[512, 4], [4, 32], [1, 4]])
                blk = (c * 16 + q4 * 4 + fx) * 4
                ENG[k % 2].dma_start(out=dst, in_=attv4[:, blk:blk + 4])
                k += 1

    if STAGE < 5:
        nc.gpsimd.dma_start(out=out[:, :], in_=P['xband'][0:64, 0:4096])
        return
    # ---------- y conv ----------
    rhsY = pool.tile([81, NPX], BF16)
    for jp in range(4):
        ENG[jp % 2].dma_start(
            out=rhsY[jp * 4:(jp + 1) * 4, :],
            in_=att_full[:, 32 * jp:32 * jp + 37, :].rearrange("f r w -> f (r w)"))
    nc.sync.dma_start(out=rhsY[16:17, :], in_=P["ones37"][:, :])
    nc.sync.dma_start(out=rhsY[17:49, :], in_=P["xband"][0:32, :])
    nc.scalar.dma_start(out=rhsY[49:81, :], in_=P["xband"][32:64, :])
    y_sb = pool.tile([65, NPX], BF16)
    nc.scalar.dma_start(out=y_sb[64:65, :], in_=P["ones37"][:, :])
    with tc.tile_pool(name="psY", bufs=2, space="PSUM") as psY:
        for n in range(10):
            n0 = n * 512
            n1 = min(n0 + 512, NPX)
            y_ps = psY.tile([64, 512], F32, name=f"yps{n}", tag="yps")
            nc.tensor.matmul(out=y_ps[:, :n1 - n0], lhsT=wcy[:, :],
                             rhs=rhsY[:, n0:n1], start=True, stop=True)
            nc.vector.tensor_copy(out=y_sb[0:64, n0:n1], in_=y_ps[:, :n1 - n0])

    if STAGE < 6:
        nc.gpsimd.dma_start(out=out[:, :], in_=P['xband'][0:64, 0:4096])
        return
    # ---------- wf1 + relu -> h1c3 ----------
    h1c3 = pool.tile([49, NROW * PITCH], BF16)
    nc.vector.memset(fv(h1c3[0:48, 0:1], [[PITCH, NROW], [1, 2]]), 0.0)
    nc.vector.memset(fv(h1c3[0:48, 130:131], [[PITCH, NROW], [1, 2]]), 0.0)
    nc.vector.memset(h1c3[32:49, :], 1.0)  # rows 32..47 overwritten below
    with tc.tile_pool(name="psH1", bufs=2, space="PSUM") as psH1:
        for n in range(10):
            n0 = n * 512
            n1 = min(n0 + 512, NPX)
            h1_ps = psH1.tile([16, 512], F32, name=f"h1ps{n}", tag="h1ps")
            nc.tensor.matmul(out=h1_ps[:, :n1 - n0], lhsT=wf1t[:, :],
                             rhs=y_sb[:, n0:n1], start=True, stop=True)
            nr = (n1 - n0) // 128
            dst = fv(h1c3[0:16, (n0 // 128) * PITCH + 2:(n0 // 128) * PITCH + 3],
                     [[PITCH, nr], [1, 128]])
            nc.vector.tensor_relu(
                out=dst, in_=h1_ps.rearrange("p (r w) -> p r w", w=128)[:, :nr, :])
    L1 = NROW * PITCH
    nc.sync.dma_start(out=fv(h1c3[16:32, 0:1], [[1, L1 - 2]]),
                      in_=fv(h1c3[0:16, 2:3], [[1, L1 - 2]]))
    nc.scalar.dma_start(out=fv(h1c3[32:48, 2:3], [[1, L1 - 2]]),
                        in_=fv(h1c3[0:16, 0:1], [[1, L1 - 2]]))

    # ---------- h2 ----------
    h2r = pool.tile([17, 32 * 128], BF16)
    nc.vector.memset(h2r[:, :], 1.0)  # row 16 stays ones; rows 0..15 overwritten
    with tc.tile_pool(name="psH2", bufs=2, space="PSUM") as psH2:
        for n in range(8):
            h2_ps = psH2.tile([16, 512], F32, name=f"h2ps{n}", tag="h2ps")
            for ky in range(3):
                base = (2 * ky + 4 * n) * PITCH + 2
                rhs = fv(h1c3[0:49, base:base + 1], [[PITCH, 4], [1, 128]])
                nc.tensor.matmul(out=h2_ps[:, :], lhsT=wf2a[:, ky * 16:(ky + 1) * 16],
                                 rhs=rhs, start=(ky == 0), stop=(ky == 2))
            nc.vector.tensor_relu(out=h2r[0:16, n * 512:(n + 1) * 512],
                                  in_=h2_ps[:, :])

    # ---------- wf3 + residual + overlapped store ----------
    out_t = pool.tile([64, 32 * 128], F32)
    with tc.tile_pool(name="psO", bufs=2, space="PSUM") as psO:
        for n in range(8):
            o_ps = psO.tile([64, 512], F32, name=f"ops{n}", tag="ops")
            nc.tensor.matmul(out=o_ps[:, :], lhsT=wf3t[:, :],
                             rhs=h2r[:, n * 512:(n + 1) * 512],
                             start=True, stop=True)
            nc.vector.tensor_add(out=out_t[:, n * 512:(n + 1) * 512],
                                 in0=o_ps[:, :],
                                 in1=y_sb[0:64, 640 + n * 512:640 + (n + 1) * 512])
            ENG[n % 2].dma_start(out=out[:, n * 512:(n + 1) * 512],
                                 in_=out_t[:, n * 512:(n + 1) * 512])


def _host_prep(inputs):
    x = np.asarray(inputs["x"], np.float32)
    g32 = lambda k: np.asarray(inputs[k], np.float32)
    w1a, b1a, w1b, b1b = g32("w1a"), g32("b1a"), g32("w1b"), g32("b1b")
    wqk = g32("wqk")
    w2, b2, wc, bc = g32("w2"), g32("b2"), g32("wc"), g32("bc")
    wf1, bf1, wf2, bf2, wf3, bf3 = (g32("wf1"), g32("bf1"), g32("wf2"),
                                    g32("bf2"), g32("wf3"), g32("bf3"))

    Wa = (w1b @ w1a).astype(np.float32)
    ba = (w1b @ b1a + b1b).astype(np.float32)
    Wc2 = (wc @ w2).astype(np.float32)
    bc2 = (wc @ b2 + bc).astype(np.float32)
    s_col = wqk.sum(axis=0).astype(np.float32)

    waq = np.zeros((128, 128), np.float32)
    for q in range(4):
        for hb in range(8):
            waq[hb * 16:(hb + 1) * 16, q * 32 + hb * 4:q * 32 + hb * 4 + 4] = \
                Wa[:, 16 * q:16 * q + 16].T
    ba32 = np.tile(ba, 8).reshape(32, 1).astype(np.float32)

    wqk20 = np.zeros((1, 20), np.float32)
    sneg20 = np.zeros((1, 20), np.float32)
    for k in range(4):
        wqk20[0, k * 5:k * 5 + 4] = wqk[:, k]
        sneg20[0, k * 5 + 4] = -s_col[k]
    svec = s_col.reshape(1, 4).astype(np.float32)

    wd2 = np.concatenate([[1.0], np.full(4, 1.0 / S),
                          np.full(16, 1.0 / (2.0 * S * S))]).astype(np.float32)
    wd2p126 = np.tile(wd2, 6).reshape(126, 1).astype(np.float32)
    pad_rhs = np.zeros((1, 96), np.float32)
    diagmask = np.zeros((126, 96), np.float32)
    for bi, (g0, gsz) in enumerate(GB):
        for gi in range(gsz):
            pad_rhs[0, bi * 30 + gi * 5 + 4] = 512.0
            diagmask[gi * 21:(gi + 1) * 21,
                     bi * 30 + gi * 5:bi * 30 + gi * 5 + 5] = 1.0
    ident = np.eye(128, dtype=np.float32)

    wf1t = np.zeros((65, 16), np.float32)
    wf1t[:64] = wf1.T
    wf1t[64] = bf1
    # h1c3 blocks: r=0 center (kx=1), r=1 holds h1[w+2] (kx=2), r=2 h1[w-2] (kx=0)
    wf2a = np.zeros((49, 48), np.float32)
    for ky in range(3):
        for kx in range(3):
            blk = (kx + 2) % 3
            wf2a[blk * 16:(blk + 1) * 16, ky * 16:(ky + 1) * 16] = \
                wf2[:, :, ky, kx].T
    wf2a[48, 2 * 16:3 * 16] = bf2
    wf3t = np.zeros((17, 64), np.float32)
    wf3t[:16] = wf3.T
    wf3t[16] = bf3

    common = dict(
        waq=waq.astype(BF), ba32=ba32,
        wqk20=wqk20, sneg20=sneg20, svec=svec,
        wd2p126=wd2p126, pad_rhs=pad_rhs, ident=ident.astype(BF),
        diagmask=diagmask.astype(BF),
        wf1t=wf1t.astype(BF), wf2a=wf2a.astype(BF), wf3t=wf3t.astype(BF))

    in_maps = []
    for i in range(8):
        b, j = i // 4, i % 4
        m = dict(common)
        m["xb"] = np.ascontiguousarray(x[b]).astype(BF)
        xband = np.zeros((64, 37, 128), np.float32)
        ones37 = np.ones((1, 37, 128), np.float32)
        if j == 0:
            xband[:, 5:] = x[b][:, 0:32]
            ones37[:, :5] = 0.0
        else:
            xband[:] = x[b][:, 32 * j - 5:32 * j + 32]
        m["xband"] = xband.reshape(64, NPX).astype(BF)
        m["ones37"] = ones37.reshape(1, NPX).astype(BF)
        wcy = np.zeros((81, 64), np.float32)
        wcy[4 * j:4 * j + 4, :] = Wc2.T
        wcy[16, :] = bc2
        wcy[17:81, :] = np.eye(64, dtype=np.float32)
        m["wcy"] = wcy.astype(BF)
        in_maps.append(m)
    return in_maps


def kernel(**inputs):
    if "nc" not in _CACHED:
        _CACHED["nc"] = build_nc()
    nc = _CACHED["nc"]
    in_maps = _host_prep(inputs)
    res = run_bass_kernel_spmd(nc, in_maps, list(range(8)))
    _CACHED["last_result"] = res
    out = np.zeros((B, C, H, W), np.float32)
    for i in range(8):
        b, j = i // 4, i % 4
        out[b, :, 32 * j:32 * j + 32, :] = \
            res.results[i]["out"].reshape(64, 32, 128)
    return out
